# revision 1
# baseline (speedup 1.0000x reference)
"""Trainium2 Bass kernel for nn_LocationEffect (GAT + temporal sigmoid attention).

out[s2*N+b, t1*N+a] = sw[b, t1, s2] * adj[b, a]
where sw = sigmoid(scale * nf nf^T per node), nf = GAT(raw_features, adj).

Sharding: row-shard the [12000, 12000] output over the node dim b.
Each of the 8 cores owns B = N/8 = 125 nodes: it computes the GAT for its
125 query rows (keys/values = all 1000 nodes, replicated), its sw slice
[125, 12, 12], and writes a [12, 125, 12000] output slab (72 MB/core).

The output is written as 144 [125, 1000] blocks, one per (s2, t1) pair.
Block (s2, t1) only depends on sw[:, t1, s2], which is ready right after
GAT timestamp max(s2, t1) — so output DMA (the memory-bound part, ~200 us
at HBM roofline) starts while the GAT is still running and hides it.
"""

import sys

import numpy as np

if "/opt/trn_rl_repo" not in sys.path:
    sys.path.insert(0, "/opt/trn_rl_repo")

T, N, D = 12, 1000, 64
NCORES = 8
B = N // NCORES  # 125 nodes per core
C = 8  # n-chunks of size B for K-tiling / transposes
NPAIR = T * (T + 1) // 2  # symmetric (t1 <= s2) pairs

_CACHE = {}


def pidx(a, b):
    """s2-major triangular index of the unordered pair {a, b}."""
    lo, hi = min(a, b), max(a, b)
    return hi * (hi + 1) // 2 + lo


def _build(repeat=1, parts="all"):
    import concourse.bacc as bacc
    import concourse.mybir as mybir
    import concourse.tile as tile
    from concourse.masks import make_identity

    f32 = mybir.dt.float32
    i32 = mybir.dt.int32
    Act = mybir.ActivationFunctionType
    Alu = mybir.AluOpType

    nc = bacc.Bacc(
        "TRN2",
        target_bir_lowering=False,
        debug=False,
        enable_asserts=True,
        num_devices=NCORES,
    )
    rf = nc.dram_tensor("rf", (T, N, D), f32, kind="ExternalInput").ap()
    rfq = nc.dram_tensor("rfq", (T, B, D), f32, kind="ExternalInput").ap()
    adj = nc.dram_tensor("adj", (B, N), i32, kind="ExternalInput").ap()
    out = nc.dram_tensor("out", (T, B, T * N), f32, kind="ExternalOutput").ap()

    with tile.TileContext(nc) as tc:
        with (
            tc.tile_pool(name="const", bufs=1) as consts,
            tc.tile_pool(name="main", bufs=1) as main,
            tc.tile_pool(name="ktp", bufs=2) as ktp,
            tc.tile_pool(name="expp", bufs=2) as expp,
            tc.tile_pool(name="swp", bufs=2) as swp,
            tc.tile_pool(name="blkp", bufs=int(__import__("os").environ.get("K_BLKBUFS", "12"))) as blkp,
            tc.tile_pool(name="ps1", bufs=1, space="PSUM") as ps1,
            tc.tile_pool(name="ps2", bufs=2, space="PSUM") as ps2,
        ):
            ident = consts.tile([128, 128], f32)
            make_identity(nc, ident[:])

            rfq_sb = main.tile([B, T, D], f32)
            nc.scalar.dma_start(out=rfq_sb[:], in_=rfq.rearrange("t p d -> p t d"))
            adj_i = main.tile([B, N], i32)
            nc.scalar.dma_start(out=adj_i[:], in_=adj)
            adjf = main.tile([B, N], f32)
            nc.vector.tensor_copy(adjf[:], adj_i[:])
            # mask in the interleaved m-order used by h_nat chunks (see below):
            # adjp[b, c*B + p] = adjf[b, p*C + c]
            adjp = main.tile([B, N], f32)
            nc.vector.tensor_copy(
                adjp[:].rearrange("b (c p) -> b c p", c=C),
                adjf[:].rearrange("b (p c) -> b c p", c=C),
            )

            # h with interleaved node chunks: chunk c = nodes {p*C + c}, so
            # partition p holds rows p*C..p*C+7 of rf[t] — 2 KB contiguous
            # per (t, p) descriptor instead of 256 B.
            h_nat = main.tile([B, T, C, D], f32)
            rf_r = rf.rearrange("t (p c) d -> p t c d", p=B)
            for t in range(T):
                nc.scalar.dma_start(out=h_nat[:, t, :, :], in_=rf_r[:, t, :, :])

            nf = main.tile([B, T, D], f32)  # normalized node features
            den = main.tile([B, T], f32)
            invden = main.tile([B, T], f32)
            swdot = main.tile([B, NPAIR], f32)
            sw = main.tile([B, NPAIR], f32)

            nblk = 0

            import os

            MUL_MIX = os.environ.get("K_MUL_MIX", "v")  # engines cycled for muls

            def emit_block(s2, t1):
                """out[s2, :, t1*N:(t1+1)*N] = adjf * sw[:, pidx]."""
                nonlocal nblk
                col = pidx(t1, s2)
                bl = blkp.tile([B, N], f32, name="bl")
                eng = MUL_MIX[nblk % len(MUL_MIX)]
                if eng == "v":
                    nc.vector.tensor_scalar_mul(bl[:], adjf[:], sw[:, col : col + 1])
                elif eng == "a":
                    nc.scalar.mul(bl[:], adjf[:], sw[:, col : col + 1])
                else:  # "p" -> gpsimd
                    nc.gpsimd.tensor_scalar_mul(bl[:], adjf[:], sw[:, col : col + 1])
                nc.sync.dma_start(out=out[s2, :, t1 * N : (t1 + 1) * N], in_=bl[:])
                nblk += 1

            for _rep in range(repeat):
                for t in range(T if parts != "p4" else 0):
                    # ---- GAT timestamp t ----
                    kt_ps = ps1.tile([64, C, 128], f32, name="kt_ps")  # 2 banks
                    for c in range(C):
                        nc.tensor.transpose(
                            kt_ps[:, c, 0:B], h_nat[:, t, c, :], ident[0:B, 0:B]
                        )
                    q_ps = ps1.tile([64, 128], f32, name="q_ps")  # 1 bank
                    nc.tensor.transpose(q_ps[:, 0:B], rfq_sb[:, t, :], ident[0:B, 0:B])
                    keysT = ktp.tile([64, C, B], f32, name="keysT")
                    nc.scalar.copy(keysT[:], kt_ps[:, :, 0:B])
                    qT = ktp.tile([64, B], f32, name="qT")
                    nc.scalar.copy(qT[:], q_ps[:, 0:B])

                    keysT_flat = keysT[:].rearrange("d c p -> d (c p)")
                    # raw scores -> exp(0.125 * scores). No max-subtraction:
                    # scaled scores <= ~15 for these inputs, exp stays finite
                    # and the softmax ratio is shift-invariant.
                    exps = expp.tile([B, N], f32, name="exps")
                    for half in range(2):
                        sc_ps = ps2.tile([B, 512], f32, name="sc_ps")  # 1 bank x2
                        nc.tensor.matmul(
                            sc_ps[:, 0:500],
                            qT[:],
                            keysT_flat[:, half * 500 : (half + 1) * 500],
                            start=True,
                            stop=True,
                        )
                        nc.scalar.activation(
                            exps[:, half * 500 : (half + 1) * 500],
                            sc_ps[:, 0:500],
                            Act.Exp,
                            scale=0.125,
                        )
                    # masked exp (adj gate) + row-sum, in one DVE op
                    mexp = expp.tile([B, N], f32, name="mexp")
                    nc.vector.scalar_tensor_tensor(
                        out=mexp[:],
                        in0=exps[:],
                        scalar=1.0,
                        in1=adjp[:],
                        op0=Alu.mult,
                        op1=Alu.mult,
                        accum_out=den[:, t : t + 1],
                    )
                    # attn^T chunks via PE transposes
                    at_ps = ps1.tile([B, C, 128], f32, name="at_ps")  # 2 banks
                    for c in range(C):
                        nc.tensor.transpose(
                            at_ps[:, c, 0:B],
                            mexp[:, c * B : (c + 1) * B],
                            ident[0:B, 0:B],
                        )
                    attnT = ktp.tile([B, C, B], f32, name="attnT")
                    nc.scalar.copy(attnT[:], at_ps[:, :, 0:B])
                    # nf_unnorm = attn^T.T @ h, K-accumulated over 8 chunks
                    nf_ps = ps1.tile([B, 64], f32, name="nf_ps")  # 1 bank
                    for c in range(C):
                        nc.tensor.matmul(
                            nf_ps[:],
                            attnT[:, c, :],
                            h_nat[:, t, c, :],
                            start=(c == 0),
                            stop=(c == C - 1),
                        )
                    nc.vector.reciprocal(invden[:, t : t + 1], den[:, t : t + 1])
                    # normalize while moving PSUM -> SBUF
                    nc.vector.tensor_scalar_mul(
                        nf[:, t, :], nf_ps[:], invden[:, t : t + 1]
                    )

                    # ---- sw pairs {t1 <= t, t}: ready now ----
                    seg = t * (t + 1) // 2
                    for t1 in range(t + 1):
                        prod = swp.tile([B, D], f32, name="prod")
                        nc.vector.scalar_tensor_tensor(
                            out=prod[:],
                            in0=nf[:, t1, :],
                            scalar=1.0,
                            in1=nf[:, t, :],
                            op0=Alu.mult,
                            op1=Alu.mult,
                            accum_out=swdot[:, seg + t1 : seg + t1 + 1],
                        )
                    nc.scalar.activation(
                        sw[:, seg : seg + t + 1],
                        swdot[:, seg : seg + t + 1],
                        Act.Sigmoid,
                        scale=0.125,
                    )

                    # ---- output blocks unlocked by timestamp t ----
                    if parts != "gat":
                        for t1 in range(t + 1):
                            emit_block(t, t1)  # tile s2 = t, columns t1 <= t
                        for s2 in range(t):
                            emit_block(s2, t)  # column t1 = t of earlier tiles

    nc.compile()
    return nc


def _get_nc(repeat=1, parts="all"):
    key = ("nc", repeat, parts)
    if key not in _CACHE:
        _CACHE[key] = _build(repeat, parts)
    return _CACHE[key]


def kernel(raw_features, adj):
    from concourse.bass_utils import run_bass_kernel_spmd

    rf = np.ascontiguousarray(np.asarray(raw_features, dtype=np.float32))
    adj_np = np.ascontiguousarray(np.asarray(adj, dtype=np.int32))

    nc = _get_nc()
    in_maps = []
    for k in range(NCORES):
        sl = slice(k * B, (k + 1) * B)
        in_maps.append(
            {
                "rf": rf,
                "rfq": np.ascontiguousarray(rf[:, sl, :]),
                "adj": np.ascontiguousarray(adj_np[sl, :]),
            }
        )
    res = run_bass_kernel_spmd(nc, in_maps, core_ids=list(range(NCORES)))
    out = np.empty((T * N, T * N), dtype=np.float32)
    ov = out.reshape(T, NCORES, B, T * N)
    for k in range(NCORES):
        ov[:, k] = res.results[k]["out"].reshape(T, B, T * N)
    return out



# revision 12
# speedup vs baseline: 1.8470x; 1.8470x over previous
"""Trainium2 Bass kernel for nn_LocationEffect (GAT + temporal sigmoid attention).

out[s2*N+b, t1*N+a] = sw[b, t1, s2] * adj[b, a]
where sw = sigmoid(scale * nf nf^T per node), nf = GAT(raw_features, adj).

Sharding: row-shard the [12000, 12000] output over the node dim b.
Each of the 8 cores owns B = N/8 = 125 nodes: it computes the GAT for its
125 query rows (keys/values = all 1000 nodes, replicated), its sw slice
[125, 12, 12], and writes a [12, 125, 12000] output slab.

Precision strategy (tolerance is 2e-2): inputs are fed to the device as
bf16 (features) / f16 (mask), the GAT attention runs in bf16 on the PE
(1 cycle/row vs 4 for fp32 matmuls), softmax statistics and node features
stay f32, and the output is written in float16, which halves HBM write
traffic: 36 MB/core -> ~100 us at the 360 GB/s cost-model roofline.
Blocks are computed on DVE (f16 hits the 4x DVE mode) and written as two
batched DMAs per timestamp t: the new row tile s2 = t (columns t1 <= t)
and the new column t1 = t of earlier row tiles (s2 < t), so output DMA
starts while the GAT is still running and hides it.
"""

import sys

import numpy as np

if "/opt/trn_rl_repo" not in sys.path:
    sys.path.insert(0, "/opt/trn_rl_repo")

T, N, D = 12, 1000, 64
NCORES = 8
B = N // NCORES  # 125 nodes per core
C = 8  # n-chunks of size B for K-tiling / transposes
NPAIR = T * (T + 1) // 2  # symmetric (t1 <= s2) pairs
NWARM = 24  # PE warmup transposes (p-state ramp)

_CACHE = {}


def pidx(a, b):
    """s2-major triangular index of the unordered pair {a, b}."""
    lo, hi = min(a, b), max(a, b)
    return hi * (hi + 1) // 2 + lo


def _build(repeat=1, parts="all"):
    import concourse.bacc as bacc
    import concourse.mybir as mybir
    import concourse.tile as tile
    from concourse.masks import make_identity

    f32 = mybir.dt.float32
    f16 = mybir.dt.float16
    bf16 = mybir.dt.bfloat16
    Act = mybir.ActivationFunctionType
    Alu = mybir.AluOpType

    nc = bacc.Bacc(
        "TRN2",
        target_bir_lowering=False,
        debug=False,
        enable_asserts=True,
        num_devices=NCORES,
    )
    rf = nc.dram_tensor("rf", (T, N, D), bf16, kind="ExternalInput").ap()
    rfq = nc.dram_tensor("rfq", (B, T, D), bf16, kind="ExternalInput").ap()
    adj = nc.dram_tensor("adj", (B, N), f16, kind="ExternalInput").ap()
    out = nc.dram_tensor("out", (T, B, T * N), f16, kind="ExternalOutput").ap()
    # b-major view for the strided column-slab DMA
    out_bsm = out.rearrange("s b m -> b s m")

    with tile.TileContext(nc) as tc:
        with (
            tc.tile_pool(name="const", bufs=1) as consts,
            tc.tile_pool(name="main", bufs=1) as main,
            tc.tile_pool(name="ktp", bufs=2) as ktp,
            tc.tile_pool(name="expp", bufs=2) as expp,
            tc.tile_pool(name="swp", bufs=2) as swp,
            tc.tile_pool(name="rowp", bufs=2) as rowp,
            tc.tile_pool(name="colp", bufs=2) as colp,
            tc.tile_pool(name="ps1", bufs=1, space="PSUM") as ps1,
            tc.tile_pool(name="ps2", bufs=2, space="PSUM") as ps2,
        ):
            ident = consts.tile([128, 128], bf16)
            make_identity(nc, ident[:])


            # inputs, in first-use order: queries, h[t=0], mask on the Act
            # queue (3 dispatches, ~2 us, then Act is free for GAT t0);
            # h[t>=1] through the idle Pool engine's SWDGE path so the Act
            # sequencer never queues behind 11 DMA dispatches.
            rfq_sb = main.tile([B, T, D], bf16)
            nc.scalar.dma_start(out=rfq_sb[:], in_=rfq)
            # h with interleaved node chunks: chunk c = nodes {p*C + c}, so
            # partition p holds rows p*C..p*C+7 of rf[t] — 1 KB contiguous
            # per (t, p) descriptor.
            h_nat = main.tile([B, T, C, D], bf16)
            rf_r = rf.rearrange("t (p c) d -> p t c d", p=B)
            nc.scalar.dma_start(out=h_nat[:, 0, :, :], in_=rf_r[:, 0, :, :])
            adjh = main.tile([B, N], f16)
            nc.scalar.dma_start(out=adjh[:], in_=adj)
            for t in range(1, T):
                nc.gpsimd.dma_start(out=h_nat[:, t, :, :], in_=rf_r[:, t, :, :])

            # mask in the interleaved m-order used by h_nat chunks:
            # adjp[b, c*B + p] = adjh[b, p*C + c]
            adjp = main.tile([B, N], bf16)
            nc.vector.tensor_copy(
                adjp[:].rearrange("b (c p) -> b c p", c=C),
                adjh[:].rearrange("b (p c) -> b c p", c=C),
            )

            nf = main.tile([B, T, D], f32)  # normalized node features
            den = main.tile([B, T], f32)
            invden = main.tile([B, T], f32)
            swdot = main.tile([B, NPAIR], f32)
            swth = main.tile([B, NPAIR], f32)
            sw = main.tile([B, NPAIR], f32)

            for _rep in range(repeat):
                for t in range(T if parts != "p4" else 0):
                    # ---- GAT timestamp t ----
                    kt_ps = ps1.tile([64, C, 128], bf16, name="kt_ps")  # 1 bank
                    for c in range(C):
                        nc.tensor.transpose(
                            kt_ps[:, c, 0:B], h_nat[:, t, c, :], ident[0:B, 0:B]
                        )
                    q_ps = ps1.tile([64, 128], bf16, name="q_ps")  # 1 bank
                    nc.tensor.transpose(q_ps[:, 0:B], rfq_sb[:, t, :], ident[0:B, 0:B])
                    keysT = ktp.tile([64, C, B], bf16, name="keysT")
                    nc.scalar.copy(keysT[:], kt_ps[:, :, 0:B])
                    qT = ktp.tile([64, B], bf16, name="qT")
                    nc.scalar.copy(qT[:], q_ps[:, 0:B])

                    keysT_flat = keysT[:].rearrange("d c p -> d (c p)")
                    # raw scores -> exp(0.125 * scores). No max-subtraction:
                    # scaled scores <= ~15 for these inputs, exp stays finite
                    # and the softmax ratio is shift-invariant.
                    exps = expp.tile([B, N], bf16, name="exps")
                    for half in range(2):
                        sc_ps = ps2.tile([B, 512], f32, name="sc_ps")  # 1 bank x2
                        nc.tensor.matmul(
                            sc_ps[:, 0:500],
                            qT[:],
                            keysT_flat[:, half * 500 : (half + 1) * 500],
                            start=True,
                            stop=True,
                        )
                        nc.scalar.activation(
                            exps[:, half * 500 : (half + 1) * 500],
                            sc_ps[:, 0:500],
                            Act.Exp,
                            scale=0.125,
                        )
                    # masked exp (adj gate): plain tensor-tensor mult hits the
                    # 4x DVE mode (all-SBUF 2-byte operands). The softmax
                    # denominator comes from the ones-column matmul below.
                    mexp = expp.tile([B, N], bf16, name="mexp")
                    nc.vector.tensor_tensor(
                        out=mexp[:], in0=exps[:], in1=adjp[:], op=Alu.mult
                    )
                    # attn^T chunks via PE transposes
                    at_ps = ps1.tile([B, C, 128], bf16, name="at_ps")  # 1 bank
                    for c in range(C):
                        nc.tensor.transpose(
                            at_ps[:, c, 0:B],
                            mexp[:, c * B : (c + 1) * B],
                            ident[0:B, 0:B],
                        )
                    attnT = ktp.tile([B, C, B], bf16, name="attnT")
                    nc.scalar.copy(attnT[:], at_ps[:, :, 0:B])
                    # softmax denominator: fast-mode DVE reduce over mexp
                    nc.vector.tensor_reduce(
                        out=den[:, t : t + 1],
                        in_=mexp[:],
                        axis=mybir.AxisListType.X,
                        op=Alu.add,
                    )
                    # nf_unnorm = attn^T.T @ h, K-accumulated over 8 chunks
                    nf_ps = ps1.tile([B, 64], f32, name="nf_ps")  # 1 bank
                    for c in range(C):
                        nc.tensor.matmul(
                            nf_ps[:],
                            attnT[:, c, :],
                            h_nat[:, t, c, :],
                            start=(c == 0),
                            stop=(c == C - 1),
                        )
                    nc.vector.reciprocal(invden[:, t : t + 1], den[:, t : t + 1])
                    # normalize while moving PSUM -> SBUF
                    nc.vector.tensor_scalar_mul(
                        nf[:, t, :], nf_ps[:], invden[:, t : t + 1]
                    )

                    # ---- sw pairs {t1 <= t, t}: ready now ----
                    seg = t * (t + 1) // 2
                    for t1 in range(t + 1):
                        prod = swp.tile([B, D], f32, name="prod")
                        nc.vector.scalar_tensor_tensor(
                            out=prod[:],
                            in0=nf[:, t1, :],
                            scalar=1.0,
                            in1=nf[:, t, :],
                            op0=Alu.mult,
                            op1=Alu.mult,
                            accum_out=swdot[:, seg + t1 : seg + t1 + 1],
                        )
                    # sigmoid(x) = 0.5*(1 + tanh(x/2)): tanh shares the Act
                    # function table with exp/copy, so the engine never
                    # reloads tables (1283 ns each) mid-kernel.
                    nc.scalar.activation(
                        swth[:, seg : seg + t + 1],
                        swdot[:, seg : seg + t + 1],
                        Act.Tanh,
                        scale=0.0625,
                    )
                    nc.vector.tensor_scalar(
                        out=sw[:, seg : seg + t + 1],
                        in0=swth[:, seg : seg + t + 1],
                        scalar1=0.5,
                        scalar2=0.5,
                        op0=Alu.mult,
                        op1=Alu.add,
                    )

                    # ---- output blocks unlocked by timestamp t ----
                    if parts != "gat":
                        # row tile s2 = t: columns t1 = 0..t, one batched DMA
                        rows = rowp.tile([B, t + 1, N], f16, name="rows")
                        for t1 in range(t + 1):
                            col = pidx(t1, t)
                            nc.vector.tensor_scalar_mul(
                                rows[:, t1, :], adjh[:], sw[:, col : col + 1]
                            )
                        nc.sync.dma_start(
                            out=out[t, :, 0 : (t + 1) * N],
                            in_=rows[:].rearrange("b t n -> b (t n)"),
                        )
                        # column t1 = t of earlier row tiles s2 = 0..t-1,
                        # one strided DMA
                        if t > 0:
                            cols = colp.tile([B, t, N], f16, name="cols")
                            for s2 in range(t):
                                col = pidx(t, s2)
                                nc.vector.tensor_scalar_mul(
                                    cols[:, s2, :], adjh[:], sw[:, col : col + 1]
                                )
                            nc.sync.dma_start(
                                out=out_bsm[:, 0:t, t * N : (t + 1) * N],
                                in_=cols[:],
                            )

    nc.compile()
    return nc


def _get_nc(repeat=1, parts="all"):
    key = ("nc", repeat, parts)
    if key not in _CACHE:
        _CACHE[key] = _build(repeat, parts)
    return _CACHE[key]


def make_in_maps(rf_f32, adj_i32):
    """Per-core input dicts from the full f32/i32 host arrays."""
    import ml_dtypes

    bf16 = ml_dtypes.bfloat16
    rf16 = np.ascontiguousarray(np.asarray(rf_f32, dtype=np.float32)).astype(bf16)
    adjh = np.asarray(adj_i32).astype(np.float16)
    in_maps = []
    for k in range(NCORES):
        sl = slice(k * B, (k + 1) * B)
        in_maps.append(
            {
                "rf": rf16,
                "rfq": np.ascontiguousarray(rf16[:, sl, :].transpose(1, 0, 2)),
                "adj": np.ascontiguousarray(adjh[sl, :]),
            }
        )
    return in_maps


def kernel(raw_features, adj):
    from concourse.bass_utils import run_bass_kernel_spmd

    nc = _get_nc()
    in_maps = make_in_maps(raw_features, adj)
    res = run_bass_kernel_spmd(nc, in_maps, core_ids=list(range(NCORES)))
    out = np.empty((T * N, T * N), dtype=np.float32)
    ov = out.reshape(T, NCORES, B, T * N)
    for k in range(NCORES):
        ov[:, k] = np.asarray(res.results[k]["out"], dtype=np.float32).reshape(
            T, B, T * N
        )
    return out
